# revision 25
# baseline (speedup 1.0000x reference)
"""Trainium2 Bass kernel for the gated-attention nn.Module.

Math (per batch element b):
    deg   = rel_pos.sum(-1)                        # [N]
    gate  = sigmoid(deg * W_d + b_d)               # [N, D]
    xg    = x * gate
    qkv   = xg @ W_qkv.T + b_qkv                   # [N, 3D]
    qk, value, res = split(qkv); qk = sigmoid(qk)
    attn  = (qk @ qk.T) * scale * rel_pos          # [N, N]
    attn  = attn / (attn.sum(-1, keepdims) + 1e-6)
    out   = relu(attn @ value + res)               # [N, D]

Sharding: pure data-parallel over batch, B == 8 == n_cores, one batch
element per NeuronCore, no collectives.

Per-core schedule:
  pass A: stream rel_pos (f32, full HWDGE rate) row-tile-wise, reducing
      row sums (deg) on DVE; pipeline gate/xg and the PE transposes of
      xg plus the qkv projections in the stream's shadow.
  pass B: per query row-tile, compute attn scores on PE in natural
      orientation, fuse the rel_pos bias multiply + row-sum (+eps) in
      one tensor_tensor_reduce (rel_pos re-read as bf16 on the
      otherwise-idle SWDGE), XBAR-transpose the bf16 attn tile, and run
      the attn @ value matmuls off the transposed copy.  Row
      normalization is applied after the matmul as a per-partition
      scale, so the N x N matrix is touched exactly once per pass.
"""

import math
from contextlib import ExitStack

import numpy as np

import concourse.bass as bass
import concourse.tile as tile
from concourse import bacc, mybir
from concourse.bass import ts
from concourse.bass_utils import run_bass_kernel_spmd
from concourse.masks import make_identity

B, N, D = 8, 2048, 256
E = 3 * D  # 768
NT = N // 128  # 16 row tiles
DC = D // 128  # 2 dim chunks
NQ = N // 512  # 4 chunks of 512 along the key dim
SCALE = 1.0 / math.sqrt(32.0)
EPS = 1e-6

F32 = mybir.dt.float32
BF16 = mybir.dt.bfloat16

AL = mybir.AluOpType
AF = mybir.ActivationFunctionType


def build_kernel(ctx: ExitStack, tc: tile.TileContext, io: dict):
    nc = tc.nc
    x_d = io["x"]          # [N, D]   f32
    rel_d = io["rel_pos"]  # [N, N]   f32
    wq_d = io["W_qkv"]     # [E, D]   f32
    bq_d = io["b_qkv"]     # [E]      f32
    wd_d = io["W_d"]       # [D, 1]   f32
    bd_d = io["b_d"]       # [D]      f32
    out_d = io["out"]      # [N, D]   f32

    # ---------------- pools ----------------
    consts = ctx.enter_context(tc.tile_pool(name="consts", bufs=1))
    resid = ctx.enter_context(tc.tile_pool(name="resid", bufs=1))
    ptbuf = ctx.enter_context(tc.tile_pool(name="ptbuf", bufs=6))
    pbuf = ctx.enter_context(tc.tile_pool(name="pbuf", bufs=2))
    xbuf = ctx.enter_context(tc.tile_pool(name="xbuf", bufs=3))
    small = ctx.enter_context(tc.tile_pool(name="small", bufs=8))
    opool = ctx.enter_context(tc.tile_pool(name="opool", bufs=2))
    ps = ctx.enter_context(tc.tile_pool(name="ps", bufs=4, space="PSUM"))
    pso = ctx.enter_context(tc.tile_pool(name="pso", bufs=3, space="PSUM"))

    # ---------------- constants ----------------
    ident = consts.tile([128, 128], BF16)
    make_identity(nc, ident)

    # W_d / b_d broadcast along partitions: [128, D]
    wd_bc = consts.tile([128, D], F32)
    nc.sync.dma_start(
        out=wd_bc,
        in_=bass.AP(tensor=wd_d.tensor, offset=wd_d.offset, ap=[[0, 128], [1, D]]),
    )
    bd_bc = consts.tile([128, D], F32)
    nc.sync.dma_start(
        out=bd_bc,
        in_=bass.AP(tensor=bd_d.tensor, offset=bd_d.offset, ap=[[0, 128], [1, D]]),
    )

    # ones row + bf16 bias rows for K=1 bias-add matmuls
    ones_row = consts.tile([1, 512], BF16)
    nc.vector.memset(ones_row, 1.0)
    bq_row_f = consts.tile([1, E], F32)
    nc.sync.dma_start(
        out=bq_row_f,
        in_=bass.AP(tensor=bq_d.tensor, offset=bq_d.offset, ap=[[1, 1], [1, E]]),
    )
    bq_row = consts.tile([1, E], BF16)
    nc.vector.tensor_copy(out=bq_row, in_=bq_row_f)

    # W_qkv natural load -> bf16 -> PE-transposed WqT[dc] = W_qkv.T chunks
    wq_nat = consts.tile([128, 6, D], F32)
    nc.sync.dma_start(out=wq_nat, in_=wq_d.rearrange("(c p) d -> p c d", p=128))
    wq_nat_bf = consts.tile([128, 6, D], BF16)
    nc.vector.tensor_copy(out=wq_nat_bf, in_=wq_nat)
    wqT = [consts.tile([128, E], BF16, tag=f"wqT{dc}", name=f"wqT{dc}") for dc in range(DC)]
    for c in range(6):
        for dc in range(DC):
            pt = ps.tile([128, 128], BF16, tag="ps", name="pt_w", padded_shape=[128, 1024])
            nc.tensor.transpose(pt, wq_nat_bf[:, c, ts(dc, 128)], ident)
            nc.scalar.copy(out=wqT[dc][:, ts(c, 128)], in_=pt)

    # ---------------- resident tensors ----------------
    qkT = [resid.tile([128, N], BF16, tag=f"qkT{dc}", name=f"qkT{dc}") for dc in range(DC)]
    xgT = [resid.tile([128, N], BF16, tag=f"xgT{dc}", name=f"xgT{dc}") for dc in range(DC)]
    vp = [resid.tile([128, D], BF16, tag=f"vp{j}", name=f"vp{j}") for j in range(NT)]
    relbf = [resid.tile([128, N], BF16, tag=f"relbf{j}", name=f"relbf{j}") for j in range(NT)]
    res = [resid.tile([128, D], F32, tag=f"res{j}", name=f"res{j}") for j in range(NT)]
    deg = resid.tile([128, NT], F32)

    # ---------------- pass A: stream rel_pos f32, deg + gated x ------------
    # software-pipelined with explicit stage lag so each engine's in-order
    # stream never waits on the cross-engine chain of the same tile
    def stage_a1(i):
        # single read of rel_pos: HBM f32 -> resident SBUF bf16 cast on the
        # SWDGE; pass B reads the resident copy (no re-read, no ring pacer)
        nc.gpsimd.dma_start(out=relbf[i], in_=rel_d[ts(i, 128), :])
        nc.vector.tensor_reduce(
            out=deg[:, i : i + 1], in_=relbf[i], axis=mybir.AxisListType.X, op=AL.add
        )
        xt = xbuf.tile([128, D], F32, tag="x", name="xt")
        nc.scalar.dma_start(out=xt, in_=x_d[ts(i, 128), :])
        return xt

    def stage_a2(i, xt):
        gate = xbuf.tile([128, D], F32, tag="gate", name="gate")
        nc.vector.scalar_tensor_tensor(
            out=gate,
            in0=wd_bc,
            scalar=deg[:, i : i + 1],
            in1=bd_bc,
            op0=AL.mult,
            op1=AL.add,
        )
        nc.scalar.activation(out=gate, in_=gate, func=AF.Sigmoid)
        xg = xbuf.tile([128, D], BF16, tag="xg", name="xg")
        nc.vector.tensor_tensor(out=xg, in0=xt, in1=gate, op=AL.mult)
        for dc in range(DC):
            pt = ps.tile([128, 128], BF16, tag="ps", name="pt_xg", padded_shape=[128, 1024])
            nc.tensor.transpose(pt, xg[:, ts(dc, 128)], ident)
            nc.scalar.copy(out=xgT[dc][:, ts(i, 128)], in_=pt)

    def stage_a3(i):
        pv = ps.tile([128, 512], F32, tag="ps", name="pv")
        for dc in range(DC):
            nc.tensor.matmul(
                pv,
                lhsT=xgT[dc][:, ts(i, 128)],
                rhs=wqT[dc][:, D : 3 * D],
                start=(dc == 0),
                stop=False,
            )
        nc.tensor.matmul(
            pv, lhsT=ones_row[:, 0:128], rhs=bq_row[:, D : 3 * D], start=False, stop=True
        )
        nc.scalar.copy(out=vp[i], in_=pv[:, 0:D])
        nc.scalar.copy(out=res[i], in_=pv[:, D : 2 * D])
        if i % 4 == 3:
            g = i // 4
            for ec in range(DC):
                pq = ps.tile([128, 512], F32, tag="ps", name="pq")
                for dc in range(DC):
                    nc.tensor.matmul(
                        pq,
                        lhsT=wqT[dc][:, ts(ec, 128)],
                        rhs=xgT[dc][:, ts(g, 512)],
                        start=(dc == 0),
                        stop=False,
                    )
                nc.tensor.matmul(
                    pq,
                    lhsT=bq_row[:, ts(ec, 128)],
                    rhs=ones_row,
                    start=False,
                    stop=True,
                )
                nc.scalar.activation(
                    out=qkT[ec][:, ts(g, 512)],
                    in_=pq,
                    func=AF.Sigmoid,
                )

    xts = {}
    for i in range(NT + 2):
        if i < NT:
            xts[i] = stage_a1(i)
        if 1 <= i <= NT:
            stage_a2(i - 1, xts.pop(i - 1))
        if i >= 2:
            stage_a3(i - 2)

    # ---------------- pass B: attention, software-pipelined -----------------
    # Per query row-tile: scores on PE (N=1024 chunks), fused bias multiply +
    # row-sum on DVE, XBAR transpose of the bf16 attn tile, then attn @ value
    # with a deep lag so no engine's in-order stream ever waits on the
    # xbar/DVE chain of a recent tile.  rel_pos is re-read as plain f32 on
    # the scalar HWDGE queue (full rate, no SWDGE cast, no xbar-mode mixing
    # with the sync queue's transposes).
    NB = NQ  # 512-wide chunks (one PSUM bank each)

    def stage_b1(i):
        P = pbuf.tile([128, N], BF16, tag="P", name="P")
        zc = small.tile([128, NB], F32, tag="zc", name="zc")
        for q in range(NB):
            pa = ps.tile([128, 512], F32, tag="ps", name="pa")
            for dc in range(DC):
                nc.tensor.matmul(
                    pa,
                    lhsT=qkT[dc][:, ts(i, 128)],
                    rhs=qkT[dc][:, ts(q, 512)],
                    start=(dc == 0),
                    stop=(dc == DC - 1),
                )
            nc.vector.scalar_tensor_tensor(
                out=P[:, ts(q, 512)],
                in0=pa,
                scalar=SCALE,
                in1=relbf[i][:, ts(q, 512)],
                op0=AL.mult,
                op1=AL.mult,
                accum_out=zc[:, q : q + 1],
            )
        # PT[p, j, q] = P[q, 128j+p]: all blocks transposed in one XBAR op
        PT = ptbuf.tile([128, NT, 128], BF16, tag="PT", name="PT")
        nc.sync.dma_start(out=PT, in_=P, transpose=True)
        return PT, zc

    def stage_b2(i, PT, zc):
        po = pso.tile([128, D], F32, tag="po", name="po")
        for j in range(NT):
            nc.tensor.matmul(
                po,
                lhsT=PT[:, j, :],
                rhs=vp[j],
                start=(j == 0),
                stop=(j == NT - 1),
            )
        z = small.tile([128, 1], F32, tag="z", name="z")
        nc.vector.tensor_reduce(out=z, in_=zc, axis=mybir.AxisListType.X, op=AL.add)
        nc.vector.tensor_scalar_add(out=z, in0=z, scalar1=EPS)
        zi = small.tile([128, 1], F32, tag="zi", name="zi")
        nc.vector.reciprocal(out=zi, in_=z)
        o = opool.tile([128, D], F32, tag="o", name="o")
        nc.vector.scalar_tensor_tensor(
            out=o, in0=po, scalar=zi, in1=res[i], op0=AL.mult, op1=AL.add
        )
        nc.scalar.activation(out=o, in_=o, func=AF.Relu)
        nc.scalar.dma_start(out=out_d[ts(i, 128), :], in_=o)

    LAG = 3
    pending = {}
    for i in range(NT + LAG):
        if i < NT:
            pending[i] = stage_b1(i)
        if i >= LAG:
            stage_b2(i - LAG, *pending.pop(i - LAG))


_CACHE: dict = {}


def _get_nc():
    if "nc" in _CACHE:
        return _CACHE["nc"], _CACHE["io"]
    nc = bacc.Bacc("TRN2", target_bir_lowering=False, debug=False)
    io = {
        "x": nc.dram_tensor("x", [N, D], F32, kind="ExternalInput").ap(),
        "rel_pos": nc.dram_tensor("rel_pos", [N, N], F32, kind="ExternalInput").ap(),
        "W_qkv": nc.dram_tensor("W_qkv", [E, D], F32, kind="ExternalInput").ap(),
        "b_qkv": nc.dram_tensor("b_qkv", [E], F32, kind="ExternalInput").ap(),
        "W_d": nc.dram_tensor("W_d", [D, 1], F32, kind="ExternalInput").ap(),
        "b_d": nc.dram_tensor("b_d", [D], F32, kind="ExternalInput").ap(),
        "out": nc.dram_tensor("out", [N, D], F32, kind="ExternalOutput").ap(),
    }
    with tile.TileContext(nc) as tc:
        with ExitStack() as ctx:
            build_kernel(ctx, tc, io)
    nc.compile()
    _CACHE["nc"] = nc
    _CACHE["io"] = io
    return nc, io


def kernel(x, rel_pos, W_qkv, b_qkv, W_d, b_d, **run_kwargs):
    nc, _ = _get_nc()
    x = np.ascontiguousarray(np.asarray(x, dtype=np.float32))
    rel_pos = np.ascontiguousarray(np.asarray(rel_pos, dtype=np.float32))
    W_qkv = np.ascontiguousarray(np.asarray(W_qkv, dtype=np.float32))
    b_qkv = np.ascontiguousarray(np.asarray(b_qkv, dtype=np.float32))
    W_d = np.ascontiguousarray(np.asarray(W_d, dtype=np.float32))
    b_d = np.ascontiguousarray(np.asarray(b_d, dtype=np.float32))
    in_maps = [
        {
            "x": x[b],
            "rel_pos": rel_pos[b],
            "W_qkv": W_qkv,
            "b_qkv": b_qkv,
            "W_d": W_d,
            "b_d": b_d,
        }
        for b in range(B)
    ]
    r = run_bass_kernel_spmd(nc, in_maps, core_ids=list(range(B)), **run_kwargs)
    out = np.stack([r.results[b]["out"] for b in range(B)], axis=0)
    if run_kwargs:
        _CACHE["last_result"] = r
    return out


# revision 27
# speedup vs baseline: 1.0312x; 1.0312x over previous
"""Trainium2 Bass kernel for the gated-attention nn.Module.

Math (per batch element b):
    deg   = rel_pos.sum(-1)                        # [N]
    gate  = sigmoid(deg * W_d + b_d)               # [N, D]
    xg    = x * gate
    qkv   = xg @ W_qkv.T + b_qkv                   # [N, 3D]
    qk, value, res = split(qkv); qk = sigmoid(qk)
    attn  = (qk @ qk.T) * scale * rel_pos          # [N, N]
    attn  = attn / (attn.sum(-1, keepdims) + 1e-6)
    out   = relu(attn @ value + res)               # [N, D]

Sharding: pure data-parallel over batch, B == 8 == n_cores, one batch
element per NeuronCore, no collectives.

Per-core schedule:
  pass A: stream rel_pos (f32, full HWDGE rate) row-tile-wise, reducing
      row sums (deg) on DVE; pipeline gate/xg and the PE transposes of
      xg plus the qkv projections in the stream's shadow.
  pass B: per query row-tile, compute attn scores on PE in natural
      orientation, fuse the rel_pos bias multiply + row-sum (+eps) in
      one tensor_tensor_reduce (rel_pos re-read as bf16 on the
      otherwise-idle SWDGE), XBAR-transpose the bf16 attn tile, and run
      the attn @ value matmuls off the transposed copy.  Row
      normalization is applied after the matmul as a per-partition
      scale, so the N x N matrix is touched exactly once per pass.
"""

import math
from contextlib import ExitStack

import numpy as np

import concourse.bass as bass
import concourse.tile as tile
from concourse import bacc, mybir
from concourse.bass import ts
from concourse.bass_utils import run_bass_kernel_spmd
from concourse.masks import make_identity

B, N, D = 8, 2048, 256
E = 3 * D  # 768
NT = N // 128  # 16 row tiles
DC = D // 128  # 2 dim chunks
NQ = N // 512  # 4 chunks of 512 along the key dim
SCALE = 1.0 / math.sqrt(32.0)
EPS = 1e-6

F32 = mybir.dt.float32
BF16 = mybir.dt.bfloat16

AL = mybir.AluOpType
AF = mybir.ActivationFunctionType


def build_kernel(ctx: ExitStack, tc: tile.TileContext, io: dict):
    nc = tc.nc
    x_d = io["x"]          # [N, D]   f32
    rel_d = io["rel_pos"]  # [N, N]   f32
    wq_d = io["W_qkv"]     # [E, D]   f32
    bq_d = io["b_qkv"]     # [E]      f32
    wd_d = io["W_d"]       # [D, 1]   f32
    bd_d = io["b_d"]       # [D]      f32
    out_d = io["out"]      # [N, D]   f32

    # ---------------- pools ----------------
    consts = ctx.enter_context(tc.tile_pool(name="consts", bufs=1))
    resid = ctx.enter_context(tc.tile_pool(name="resid", bufs=1))
    ptbuf = ctx.enter_context(tc.tile_pool(name="ptbuf", bufs=6))
    pbuf = ctx.enter_context(tc.tile_pool(name="pbuf", bufs=3))
    xbuf = ctx.enter_context(tc.tile_pool(name="xbuf", bufs=4))
    small = ctx.enter_context(tc.tile_pool(name="small", bufs=8))
    opool = ctx.enter_context(tc.tile_pool(name="opool", bufs=2))
    ps = ctx.enter_context(tc.tile_pool(name="ps", bufs=4, space="PSUM"))
    pso = ctx.enter_context(tc.tile_pool(name="pso", bufs=3, space="PSUM"))

    # ---------------- constants ----------------
    ident = consts.tile([128, 128], BF16)
    make_identity(nc, ident)

    # W_d / b_d broadcast along partitions: [128, D]
    wd_bc = consts.tile([128, D], F32)
    nc.sync.dma_start(
        out=wd_bc,
        in_=bass.AP(tensor=wd_d.tensor, offset=wd_d.offset, ap=[[0, 128], [1, D]]),
    )
    bd_bc = consts.tile([128, D], F32)
    nc.sync.dma_start(
        out=bd_bc,
        in_=bass.AP(tensor=bd_d.tensor, offset=bd_d.offset, ap=[[0, 128], [1, D]]),
    )

    # ones row + bf16 bias rows for K=1 bias-add matmuls
    ones_row = consts.tile([1, 512], BF16)
    nc.vector.memset(ones_row, 1.0)
    bq_row_f = consts.tile([1, E], F32)
    nc.sync.dma_start(
        out=bq_row_f,
        in_=bass.AP(tensor=bq_d.tensor, offset=bq_d.offset, ap=[[1, 1], [1, E]]),
    )
    bq_row = consts.tile([1, E], BF16)
    nc.vector.tensor_copy(out=bq_row, in_=bq_row_f)

    # W_qkv natural load -> bf16 -> PE-transposed WqT[dc] = W_qkv.T chunks
    wq_nat = consts.tile([128, 6, D], F32)
    nc.sync.dma_start(out=wq_nat, in_=wq_d.rearrange("(c p) d -> p c d", p=128))
    wq_nat_bf = consts.tile([128, 6, D], BF16)
    nc.vector.tensor_copy(out=wq_nat_bf, in_=wq_nat)
    wqT = [consts.tile([128, E], BF16, tag=f"wqT{dc}", name=f"wqT{dc}") for dc in range(DC)]
    for c in range(6):
        for dc in range(DC):
            pt = ps.tile([128, 128], BF16, tag="ps", name="pt_w", padded_shape=[128, 1024])
            nc.tensor.transpose(pt, wq_nat_bf[:, c, ts(dc, 128)], ident)
            nc.scalar.copy(out=wqT[dc][:, ts(c, 128)], in_=pt)

    # ---------------- resident tensors ----------------
    qkT = [resid.tile([128, N], BF16, tag=f"qkT{dc}", name=f"qkT{dc}") for dc in range(DC)]
    xgT = [resid.tile([128, N], BF16, tag=f"xgT{dc}", name=f"xgT{dc}") for dc in range(DC)]
    vp = [resid.tile([128, D], BF16, tag=f"vp{j}", name=f"vp{j}") for j in range(NT)]
    relbf = [resid.tile([128, N], BF16, tag=f"relbf{j}", name=f"relbf{j}") for j in range(NT)]
    res = [resid.tile([128, D], F32, tag=f"res{j}", name=f"res{j}") for j in range(NT)]
    deg = resid.tile([128, NT], F32)

    # ---------------- pass A: stream rel_pos f32, deg + gated x ------------
    # software-pipelined with explicit stage lag so each engine's in-order
    # stream never waits on the cross-engine chain of the same tile
    def stage_a1(i):
        # single read of rel_pos: HBM f32 -> resident SBUF bf16 cast on the
        # SWDGE; pass B reads the resident copy (no re-read, no ring pacer)
        nc.gpsimd.dma_start(out=relbf[i], in_=rel_d[ts(i, 128), :])
        nc.vector.tensor_reduce(
            out=deg[:, i : i + 1], in_=relbf[i], axis=mybir.AxisListType.X, op=AL.add
        )
        xt = xbuf.tile([128, D], F32, tag="x", name="xt")
        nc.scalar.dma_start(out=xt, in_=x_d[ts(i, 128), :])
        return xt

    def stage_a2(i, xt):
        gate = xbuf.tile([128, D], F32, tag="gate", name="gate")
        nc.vector.scalar_tensor_tensor(
            out=gate,
            in0=wd_bc,
            scalar=deg[:, i : i + 1],
            in1=bd_bc,
            op0=AL.mult,
            op1=AL.add,
        )
        nc.scalar.activation(out=gate, in_=gate, func=AF.Sigmoid)
        xg = xbuf.tile([128, D], BF16, tag="xg", name="xg")
        nc.vector.tensor_tensor(out=xg, in0=xt, in1=gate, op=AL.mult)
        for dc in range(DC):
            pt = ps.tile([128, 128], BF16, tag="ps", name="pt_xg", padded_shape=[128, 1024])
            nc.tensor.transpose(pt, xg[:, ts(dc, 128)], ident)
            nc.scalar.copy(out=xgT[dc][:, ts(i, 128)], in_=pt)

    def stage_a3(i):
        pv = ps.tile([128, 512], F32, tag="ps", name="pv")
        for dc in range(DC):
            nc.tensor.matmul(
                pv,
                lhsT=xgT[dc][:, ts(i, 128)],
                rhs=wqT[dc][:, D : 3 * D],
                start=(dc == 0),
                stop=False,
            )
        nc.tensor.matmul(
            pv, lhsT=ones_row[:, 0:128], rhs=bq_row[:, D : 3 * D], start=False, stop=True
        )
        nc.scalar.copy(out=vp[i], in_=pv[:, 0:D])
        nc.scalar.copy(out=res[i], in_=pv[:, D : 2 * D])
        if i % 4 == 3:
            g = i // 4
            for ec in range(DC):
                pq = ps.tile([128, 512], F32, tag="ps", name="pq")
                for dc in range(DC):
                    nc.tensor.matmul(
                        pq,
                        lhsT=wqT[dc][:, ts(ec, 128)],
                        rhs=xgT[dc][:, ts(g, 512)],
                        start=(dc == 0),
                        stop=False,
                    )
                nc.tensor.matmul(
                    pq,
                    lhsT=bq_row[:, ts(ec, 128)],
                    rhs=ones_row,
                    start=False,
                    stop=True,
                )
                nc.scalar.activation(
                    out=qkT[ec][:, ts(g, 512)],
                    in_=pq,
                    func=AF.Sigmoid,
                )

    xts = {}
    for i in range(NT + 2):
        if i < NT:
            xts[i] = stage_a1(i)
        if 1 <= i <= NT:
            stage_a2(i - 1, xts.pop(i - 1))
        if i >= 2:
            stage_a3(i - 2)

    # ---------------- pass B: attention, software-pipelined -----------------
    # Per query row-tile: scores on PE (N=1024 chunks), fused bias multiply +
    # row-sum on DVE, XBAR transpose of the bf16 attn tile, then attn @ value
    # with a deep lag so no engine's in-order stream ever waits on the
    # xbar/DVE chain of a recent tile.  rel_pos is re-read as plain f32 on
    # the scalar HWDGE queue (full rate, no SWDGE cast, no xbar-mode mixing
    # with the sync queue's transposes).
    NB = NQ  # 512-wide chunks (one PSUM bank each)

    def stage_b1(i):
        P = pbuf.tile([128, N], BF16, tag="P", name="P")
        zc = small.tile([128, NB], F32, tag="zc", name="zc")
        for q in range(NB):
            pa = ps.tile([128, 512], F32, tag="ps", name="pa")
            for dc in range(DC):
                nc.tensor.matmul(
                    pa,
                    lhsT=qkT[dc][:, ts(i, 128)],
                    rhs=qkT[dc][:, ts(q, 512)],
                    start=(dc == 0),
                    stop=(dc == DC - 1),
                )
            nc.vector.scalar_tensor_tensor(
                out=P[:, ts(q, 512)],
                in0=pa,
                scalar=SCALE,
                in1=relbf[i][:, ts(q, 512)],
                op0=AL.mult,
                op1=AL.mult,
                accum_out=zc[:, q : q + 1],
            )
        # PT[p, j, q] = P[q, 128j+p]: all blocks transposed in one XBAR op
        PT = ptbuf.tile([128, NT, 128], BF16, tag="PT", name="PT")
        nc.sync.dma_start(out=PT, in_=P, transpose=True)
        return PT, zc

    def stage_b2(i, PT, zc):
        po = pso.tile([128, D], F32, tag="po", name="po")
        for j in range(NT):
            nc.tensor.matmul(
                po,
                lhsT=PT[:, j, :],
                rhs=vp[j],
                start=(j == 0),
                stop=(j == NT - 1),
            )
        z = small.tile([128, 1], F32, tag="z", name="z")
        nc.vector.tensor_reduce(out=z, in_=zc, axis=mybir.AxisListType.X, op=AL.add)
        nc.vector.tensor_scalar_add(out=z, in0=z, scalar1=EPS)
        zi = small.tile([128, 1], F32, tag="zi", name="zi")
        nc.vector.reciprocal(out=zi, in_=z)
        o = opool.tile([128, D], F32, tag="o", name="o")
        nc.vector.scalar_tensor_tensor(
            out=o, in0=po, scalar=zi, in1=res[i], op0=AL.mult, op1=AL.add
        )
        nc.scalar.activation(out=o, in_=o, func=AF.Relu)
        nc.scalar.dma_start(out=out_d[ts(i, 128), :], in_=o)

    LAG = 3
    pending = {}
    for i in range(NT + LAG):
        if i < NT:
            pending[i] = stage_b1(i)
        if i >= LAG:
            stage_b2(i - LAG, *pending.pop(i - LAG))


_CACHE: dict = {}


def _get_nc():
    if "nc" in _CACHE:
        return _CACHE["nc"], _CACHE["io"]
    nc = bacc.Bacc("TRN2", target_bir_lowering=False, debug=False)
    io = {
        "x": nc.dram_tensor("x", [N, D], F32, kind="ExternalInput").ap(),
        "rel_pos": nc.dram_tensor("rel_pos", [N, N], F32, kind="ExternalInput").ap(),
        "W_qkv": nc.dram_tensor("W_qkv", [E, D], F32, kind="ExternalInput").ap(),
        "b_qkv": nc.dram_tensor("b_qkv", [E], F32, kind="ExternalInput").ap(),
        "W_d": nc.dram_tensor("W_d", [D, 1], F32, kind="ExternalInput").ap(),
        "b_d": nc.dram_tensor("b_d", [D], F32, kind="ExternalInput").ap(),
        "out": nc.dram_tensor("out", [N, D], F32, kind="ExternalOutput").ap(),
    }
    with tile.TileContext(nc) as tc:
        with ExitStack() as ctx:
            build_kernel(ctx, tc, io)
    nc.compile()
    _CACHE["nc"] = nc
    _CACHE["io"] = io
    return nc, io


def kernel(x, rel_pos, W_qkv, b_qkv, W_d, b_d, **run_kwargs):
    nc, _ = _get_nc()
    x = np.ascontiguousarray(np.asarray(x, dtype=np.float32))
    rel_pos = np.ascontiguousarray(np.asarray(rel_pos, dtype=np.float32))
    W_qkv = np.ascontiguousarray(np.asarray(W_qkv, dtype=np.float32))
    b_qkv = np.ascontiguousarray(np.asarray(b_qkv, dtype=np.float32))
    W_d = np.ascontiguousarray(np.asarray(W_d, dtype=np.float32))
    b_d = np.ascontiguousarray(np.asarray(b_d, dtype=np.float32))
    in_maps = [
        {
            "x": x[b],
            "rel_pos": rel_pos[b],
            "W_qkv": W_qkv,
            "b_qkv": b_qkv,
            "W_d": W_d,
            "b_d": b_d,
        }
        for b in range(B)
    ]
    r = run_bass_kernel_spmd(nc, in_maps, core_ids=list(range(B)), **run_kwargs)
    out = np.stack([r.results[b]["out"] for b in range(B)], axis=0)
    if run_kwargs:
        _CACHE["last_result"] = r
    return out


# revision 29
# speedup vs baseline: 1.0565x; 1.0245x over previous
"""Trainium2 Bass kernel for the gated-attention nn.Module.

Math (per batch element b):
    deg   = rel_pos.sum(-1)                        # [N]
    gate  = sigmoid(deg * W_d + b_d)               # [N, D]
    xg    = x * gate
    qkv   = xg @ W_qkv.T + b_qkv                   # [N, 3D]
    qk, value, res = split(qkv); qk = sigmoid(qk)
    attn  = (qk @ qk.T) * scale * rel_pos          # [N, N]
    attn  = attn / (attn.sum(-1, keepdims) + 1e-6)
    out   = relu(attn @ value + res)               # [N, D]

Sharding: pure data-parallel over batch, B == 8 == n_cores, one batch
element per NeuronCore, no collectives.

Per-core schedule:
  pass A: stream rel_pos (f32, full HWDGE rate) row-tile-wise, reducing
      row sums (deg) on DVE; pipeline gate/xg and the PE transposes of
      xg plus the qkv projections in the stream's shadow.
  pass B: per query row-tile, compute attn scores on PE in natural
      orientation, fuse the rel_pos bias multiply + row-sum (+eps) in
      one tensor_tensor_reduce (rel_pos re-read as bf16 on the
      otherwise-idle SWDGE), XBAR-transpose the bf16 attn tile, and run
      the attn @ value matmuls off the transposed copy.  Row
      normalization is applied after the matmul as a per-partition
      scale, so the N x N matrix is touched exactly once per pass.
"""

import math
from contextlib import ExitStack

import numpy as np

import concourse.bass as bass
import concourse.tile as tile
from concourse import bacc, mybir
from concourse.bass import ts
from concourse.bass_utils import run_bass_kernel_spmd
from concourse.masks import make_identity

B, N, D = 8, 2048, 256
E = 3 * D  # 768
NT = N // 128  # 16 row tiles
DC = D // 128  # 2 dim chunks
NQ = N // 512  # 4 chunks of 512 along the key dim
SCALE = 1.0 / math.sqrt(32.0)
EPS = 1e-6

F32 = mybir.dt.float32
BF16 = mybir.dt.bfloat16

AL = mybir.AluOpType
AF = mybir.ActivationFunctionType


def build_kernel(ctx: ExitStack, tc: tile.TileContext, io: dict):
    nc = tc.nc
    x_d = io["x"]          # [N, D]   f32
    rel_d = io["rel_pos"]  # [N, N]   f32
    wq_d = io["W_qkv"]     # [E, D]   f32
    bq_d = io["b_qkv"]     # [E]      f32
    wd_d = io["W_d"]       # [D, 1]   f32
    bd_d = io["b_d"]       # [D]      f32
    out_d = io["out"]      # [N, D]   f32

    # ---------------- pools ----------------
    consts = ctx.enter_context(tc.tile_pool(name="consts", bufs=1))
    resid = ctx.enter_context(tc.tile_pool(name="resid", bufs=1))
    ptbuf = ctx.enter_context(tc.tile_pool(name="ptbuf", bufs=7))
    pbuf = ctx.enter_context(tc.tile_pool(name="pbuf", bufs=3))
    xbuf = ctx.enter_context(tc.tile_pool(name="xbuf", bufs=4))
    small = ctx.enter_context(tc.tile_pool(name="small", bufs=8))
    opool = ctx.enter_context(tc.tile_pool(name="opool", bufs=2))
    ps = ctx.enter_context(tc.tile_pool(name="ps", bufs=4, space="PSUM"))
    pso = ctx.enter_context(tc.tile_pool(name="pso", bufs=3, space="PSUM"))

    # ---------------- constants ----------------
    ident = consts.tile([128, 128], BF16)
    make_identity(nc, ident)

    # W_d / b_d broadcast along partitions: [128, D]
    wd_bc = consts.tile([128, D], F32)
    nc.sync.dma_start(
        out=wd_bc,
        in_=bass.AP(tensor=wd_d.tensor, offset=wd_d.offset, ap=[[0, 128], [1, D]]),
    )
    bd_bc = consts.tile([128, D], F32)
    nc.sync.dma_start(
        out=bd_bc,
        in_=bass.AP(tensor=bd_d.tensor, offset=bd_d.offset, ap=[[0, 128], [1, D]]),
    )

    # ones row + bf16 bias rows for K=1 bias-add matmuls
    ones_row = consts.tile([1, 512], BF16)
    nc.vector.memset(ones_row, 1.0)
    bq_row_f = consts.tile([1, E], F32)
    nc.sync.dma_start(
        out=bq_row_f,
        in_=bass.AP(tensor=bq_d.tensor, offset=bq_d.offset, ap=[[1, 1], [1, E]]),
    )
    bq_row = consts.tile([1, E], BF16)
    nc.vector.tensor_copy(out=bq_row, in_=bq_row_f)

    # W_qkv natural load -> bf16 -> PE-transposed WqT[dc] = W_qkv.T chunks
    wq_nat = consts.tile([128, 6, D], F32)
    nc.sync.dma_start(out=wq_nat, in_=wq_d.rearrange("(c p) d -> p c d", p=128))
    wq_nat_bf = consts.tile([128, 6, D], BF16)
    nc.vector.tensor_copy(out=wq_nat_bf, in_=wq_nat)
    wqT = [consts.tile([128, E], BF16, tag=f"wqT{dc}", name=f"wqT{dc}") for dc in range(DC)]
    for c in range(6):
        for dc in range(DC):
            pt = ps.tile([128, 128], BF16, tag="ps", name="pt_w", padded_shape=[128, 1024])
            nc.tensor.transpose(pt, wq_nat_bf[:, c, ts(dc, 128)], ident)
            nc.scalar.copy(out=wqT[dc][:, ts(c, 128)], in_=pt)

    # ---------------- resident tensors ----------------
    qkT = [resid.tile([128, N], BF16, tag=f"qkT{dc}", name=f"qkT{dc}") for dc in range(DC)]
    xgT = [resid.tile([128, N], BF16, tag=f"xgT{dc}", name=f"xgT{dc}") for dc in range(DC)]
    vp = [resid.tile([128, D], BF16, tag=f"vp{j}", name=f"vp{j}") for j in range(NT)]
    relbf = [resid.tile([128, N], BF16, tag=f"relbf{j}", name=f"relbf{j}") for j in range(NT)]
    res = [resid.tile([128, D], F32, tag=f"res{j}", name=f"res{j}") for j in range(NT)]
    deg = resid.tile([128, NT], F32)

    # ---------------- pass A: stream rel_pos f32, deg + gated x ------------
    # software-pipelined with explicit stage lag so each engine's in-order
    # stream never waits on the cross-engine chain of the same tile
    def stage_a1(i):
        # single read of rel_pos: HBM f32 -> resident SBUF bf16 cast on the
        # SWDGE; pass B reads the resident copy (no re-read, no ring pacer)
        nc.gpsimd.dma_start(out=relbf[i], in_=rel_d[ts(i, 128), :])
        nc.vector.tensor_reduce(
            out=deg[:, i : i + 1], in_=relbf[i], axis=mybir.AxisListType.X, op=AL.add
        )
        xt = xbuf.tile([128, D], F32, tag="x", name="xt")
        nc.scalar.dma_start(out=xt, in_=x_d[ts(i, 128), :])
        return xt

    def stage_a2(i, xt):
        gate = xbuf.tile([128, D], F32, tag="gate", name="gate")
        nc.vector.scalar_tensor_tensor(
            out=gate,
            in0=wd_bc,
            scalar=deg[:, i : i + 1],
            in1=bd_bc,
            op0=AL.mult,
            op1=AL.add,
        )
        nc.scalar.activation(out=gate, in_=gate, func=AF.Sigmoid)
        xg = xbuf.tile([128, D], BF16, tag="xg", name="xg")
        nc.vector.tensor_tensor(out=xg, in0=xt, in1=gate, op=AL.mult)
        for dc in range(DC):
            pt = ps.tile([128, 128], BF16, tag="ps", name="pt_xg", padded_shape=[128, 1024])
            nc.tensor.transpose(pt, xg[:, ts(dc, 128)], ident)
            nc.scalar.copy(out=xgT[dc][:, ts(i, 128)], in_=pt)

    def stage_a3(i):
        pv = ps.tile([128, 512], F32, tag="ps", name="pv")
        for dc in range(DC):
            nc.tensor.matmul(
                pv,
                lhsT=xgT[dc][:, ts(i, 128)],
                rhs=wqT[dc][:, D : 3 * D],
                start=(dc == 0),
                stop=False,
            )
        nc.tensor.matmul(
            pv, lhsT=ones_row[:, 0:128], rhs=bq_row[:, D : 3 * D], start=False, stop=True
        )
        nc.scalar.copy(out=vp[i], in_=pv[:, 0:D])
        nc.scalar.copy(out=res[i], in_=pv[:, D : 2 * D])
        if i % 4 == 3:
            g = i // 4
            for ec in range(DC):
                pq = ps.tile([128, 512], F32, tag="ps", name="pq")
                for dc in range(DC):
                    nc.tensor.matmul(
                        pq,
                        lhsT=wqT[dc][:, ts(ec, 128)],
                        rhs=xgT[dc][:, ts(g, 512)],
                        start=(dc == 0),
                        stop=False,
                    )
                nc.tensor.matmul(
                    pq,
                    lhsT=bq_row[:, ts(ec, 128)],
                    rhs=ones_row,
                    start=False,
                    stop=True,
                )
                nc.scalar.activation(
                    out=qkT[ec][:, ts(g, 512)],
                    in_=pq,
                    func=AF.Sigmoid,
                )

    xts = {}
    for i in range(NT + 2):
        if i < NT:
            xts[i] = stage_a1(i)
        if 1 <= i <= NT:
            stage_a2(i - 1, xts.pop(i - 1))
        if i >= 2:
            stage_a3(i - 2)

    # ---------------- pass B: attention, software-pipelined -----------------
    # Per query row-tile: scores on PE (N=1024 chunks), fused bias multiply +
    # row-sum on DVE, XBAR transpose of the bf16 attn tile, then attn @ value
    # with a deep lag so no engine's in-order stream ever waits on the
    # xbar/DVE chain of a recent tile.  rel_pos is re-read as plain f32 on
    # the scalar HWDGE queue (full rate, no SWDGE cast, no xbar-mode mixing
    # with the sync queue's transposes).
    NB = NQ  # 512-wide chunks (one PSUM bank each)

    def stage_b1(i):
        P = pbuf.tile([128, N], BF16, tag="P", name="P")
        zc = small.tile([128, NB], F32, tag="zc", name="zc")
        for q in range(NB):
            pa = ps.tile([128, 512], F32, tag="ps", name="pa")
            for dc in range(DC):
                nc.tensor.matmul(
                    pa,
                    lhsT=qkT[dc][:, ts(i, 128)],
                    rhs=qkT[dc][:, ts(q, 512)],
                    start=(dc == 0),
                    stop=(dc == DC - 1),
                )
            nc.vector.scalar_tensor_tensor(
                out=P[:, ts(q, 512)],
                in0=pa,
                scalar=SCALE,
                in1=relbf[i][:, ts(q, 512)],
                op0=AL.mult,
                op1=AL.mult,
                accum_out=zc[:, q : q + 1],
            )
        # PT[p, j, q] = P[q, 128j+p]: all blocks transposed in one XBAR op
        PT = ptbuf.tile([128, NT, 128], BF16, tag="PT", name="PT")
        nc.sync.dma_start(out=PT, in_=P, transpose=True)
        return PT, zc

    def stage_b2(i, PT, zc):
        po = pso.tile([128, D], F32, tag="po", name="po")
        for j in range(NT):
            nc.tensor.matmul(
                po,
                lhsT=PT[:, j, :],
                rhs=vp[j],
                start=(j == 0),
                stop=(j == NT - 1),
            )
        z = small.tile([128, 1], F32, tag="z", name="z")
        nc.vector.tensor_reduce(out=z, in_=zc, axis=mybir.AxisListType.X, op=AL.add)
        nc.vector.tensor_scalar_add(out=z, in0=z, scalar1=EPS)
        zi = small.tile([128, 1], F32, tag="zi", name="zi")
        nc.vector.reciprocal(out=zi, in_=z)
        o = opool.tile([128, D], F32, tag="o", name="o")
        nc.vector.scalar_tensor_tensor(
            out=o, in0=po, scalar=zi, in1=res[i], op0=AL.mult, op1=AL.add
        )
        nc.scalar.activation(out=o, in_=o, func=AF.Relu)
        nc.scalar.dma_start(out=out_d[ts(i, 128), :], in_=o)

    LAG = 6
    pending = {}
    for i in range(NT + LAG):
        if i < NT:
            pending[i] = stage_b1(i)
        if i >= LAG:
            stage_b2(i - LAG, *pending.pop(i - LAG))


_CACHE: dict = {}


def _get_nc():
    if "nc" in _CACHE:
        return _CACHE["nc"], _CACHE["io"]
    nc = bacc.Bacc("TRN2", target_bir_lowering=False, debug=False)
    io = {
        "x": nc.dram_tensor("x", [N, D], F32, kind="ExternalInput").ap(),
        "rel_pos": nc.dram_tensor("rel_pos", [N, N], F32, kind="ExternalInput").ap(),
        "W_qkv": nc.dram_tensor("W_qkv", [E, D], F32, kind="ExternalInput").ap(),
        "b_qkv": nc.dram_tensor("b_qkv", [E], F32, kind="ExternalInput").ap(),
        "W_d": nc.dram_tensor("W_d", [D, 1], F32, kind="ExternalInput").ap(),
        "b_d": nc.dram_tensor("b_d", [D], F32, kind="ExternalInput").ap(),
        "out": nc.dram_tensor("out", [N, D], F32, kind="ExternalOutput").ap(),
    }
    with tile.TileContext(nc) as tc:
        with ExitStack() as ctx:
            build_kernel(ctx, tc, io)
    nc.compile()
    _CACHE["nc"] = nc
    _CACHE["io"] = io
    return nc, io


def kernel(x, rel_pos, W_qkv, b_qkv, W_d, b_d, **run_kwargs):
    nc, _ = _get_nc()
    x = np.ascontiguousarray(np.asarray(x, dtype=np.float32))
    rel_pos = np.ascontiguousarray(np.asarray(rel_pos, dtype=np.float32))
    W_qkv = np.ascontiguousarray(np.asarray(W_qkv, dtype=np.float32))
    b_qkv = np.ascontiguousarray(np.asarray(b_qkv, dtype=np.float32))
    W_d = np.ascontiguousarray(np.asarray(W_d, dtype=np.float32))
    b_d = np.ascontiguousarray(np.asarray(b_d, dtype=np.float32))
    in_maps = [
        {
            "x": x[b],
            "rel_pos": rel_pos[b],
            "W_qkv": W_qkv,
            "b_qkv": b_qkv,
            "W_d": W_d,
            "b_d": b_d,
        }
        for b in range(B)
    ]
    r = run_bass_kernel_spmd(nc, in_maps, core_ids=list(range(B)), **run_kwargs)
    out = np.stack([r.results[b]["out"] for b in range(B)], axis=0)
    if run_kwargs:
        _CACHE["last_result"] = r
    return out
